# revision 18
# baseline (speedup 1.0000x reference)
"""3-layer GAT (negbin head) Trainium2 Bass kernel, 8-core SPMD.

Strategy (graph/data parallel, dst-sharded):
  - Nodes are sharded contiguously across 8 cores (dst ownership).
  - Per layer: each core transforms its own nodes (t = h @ W, node-major
    PE matmuls) and packs a gather-table row [h_bf16(128) | 1.0 | pad |
    a_src_f32] per node. Tables are exchanged between launches (host
    concat; collectives are unavailable through this NRT path).
  - Attention: per-edge rows are fetched with indirect DMA (one 264B
    descriptor per edge), logits z = a_src[src]+a_dst[dst] -> leaky_relu ->
    exp on DVE/ACT, exp values scattered into one-hot "St" matrices on
    GpSimd (local_scatter), and one PE matmul per 128-edge chunk computes
    both the exp-weighted feature sum and the softmax denominator (via the
    table's ones column) into PSUM. A per-tile epilogue normalizes, adds
    bias, and prepares the next layer's transform input in SBUF.
  - Edge bookkeeping (sorting by dst, chunking, padding so all cores run
    the identical SPMD program) is integer-only host preprocessing.

Four launches: [tf1] -> [att1+tf2] -> [att2+tf3] -> [att3]; the attention
layer feeding a transform stays in one launch (h2T/meanT live in SBUF).
"""
import sys

sys.path.insert(0, "/opt/trn_rl_repo")

import numpy as np
import ml_dtypes

from concourse import bass, bacc, mybir, tile
from concourse.masks import make_identity

BF16 = ml_dtypes.bfloat16

# ---- problem constants (hardcoded; harness provides full-size inputs) ----
N_NODES = 50000
N_EDGES = 600000
IN_CH = 512
CH = 128
NCORES = 8
NEG_SLOPE = 0.2

ROW_U16 = 132  # u16 slots per row: [0:128]=h bf16, 128:1.0, 129:pad, 130:132=a_src f32
LS = 14        # chunks per local_scatter group (num_elems = 14*128 = 1792 <= 2046)
GG = 42        # chunks per gather group (= 3 local_scatter groups)


def _npc():
    return N_NODES // NCORES


def _t_tiles():
    return (_npc() + 127) // 128


def _npc_pad():
    return _t_tiles() * 128


# --------------------------------------------------------------------------
# Host planning: integer-only edge bookkeeping
# --------------------------------------------------------------------------

def _plan_layer(src, dst):
    """src/dst: global node id arrays. Returns per-core slot arrays + shared
    chunk->tile map. All cores get the same G and tile map (SPMD)."""
    NPC, T, NPCP = _npc(), _t_tiles(), _npc_pad()
    per_core = []
    tile_counts = []
    k = np.zeros(T, dtype=np.int64)
    for c in range(NCORES):
        lo, hi = c * NPC, (c + 1) * NPC
        idx = np.nonzero((dst >= lo) & (dst < hi))[0]
        es = src[idx].astype(np.int64)
        ed = (dst[idx] - lo).astype(np.int64)
        order = np.argsort(ed, kind="stable")
        es, ed, idx = es[order], ed[order], idx[order]
        per_core.append((es, ed, idx))
        cnt = np.bincount(ed // 128, minlength=T)
        tile_counts.append(cnt)
        k = np.maximum(k, (cnt + 127) // 128)
    k = np.maximum(k, 1)
    G = int(k.sum())
    G_pad = ((G + LS - 1) // LS) * LS
    k[-1] += G_pad - G
    G = G_pad
    tile_of_chunk = np.repeat(np.arange(T), k)
    chunk_start = np.zeros(T, dtype=np.int64)
    chunk_start[1:] = np.cumsum(k)[:-1]

    cores = []
    for c in range(NCORES):
        es, ed, orig = per_core[c]
        src_off = np.zeros((128, G), dtype=np.int32)
        adst_off = np.zeros((128, G), dtype=np.int32)
        st_idx = np.full((128, G), -1, dtype=np.int16)
        slot_p = np.zeros(len(es), dtype=np.int64)
        slot_g = np.zeros(len(es), dtype=np.int64)
        tstarts = np.zeros(T + 1, dtype=np.int64)
        tstarts[1:] = np.cumsum(tile_counts[c])
        for t in range(T):
            e0, e1 = int(tstarts[t]), int(tstarts[t + 1])
            n = e1 - e0
            if n == 0:
                continue
            i = np.arange(n)
            g = chunk_start[t] + i // 128
            p = i % 128
            gsrc = es[e0:e1]
            src_off[p, g] = ((gsrc // NPC) * NPCP + (gsrc % NPC)).astype(np.int32)
            dl = ed[e0:e1]
            adst_off[p, g] = ((dl % 128) * T + (dl // 128)).astype(np.int32)
            st_idx[p, g] = ((g % LS) * 128 + (dl % 128)).astype(np.int16)
            slot_p[e0:e1] = p
            slot_g[e0:e1] = g
        cores.append(
            dict(src_off=src_off, adst_off=adst_off, st_idx=st_idx,
                 orig=orig, slot_p=slot_p, slot_g=slot_g)
        )
    return dict(G=G, tile_of_chunk=tile_of_chunk, k=k, cores=cores)


# --------------------------------------------------------------------------
# Stage program builders
# --------------------------------------------------------------------------
# stage 0: tf(L1)            ins: xT,W1,as1,ad1            outs: tloc,adst
# stage 1: att(L1)+tf(L2)    ins: tfull,adst,offs12,b1,W2,as2,ad2   outs: tloc,adst
# stage 2: att(L2)+tf(L3)    ins: tfull,adst,offs12,b2,Wv,asv,adv   outs: mean,alpha,tloc,adst
# stage 3: att(L3)           ins: tfull,adst,offs3,bv      outs: var

def _build_stage(stage, G, tmap):
    NPC, T, NPCP = _npc(), _t_tiles(), _npc_pad()
    f32, bf16, u16 = mybir.dt.float32, mybir.dt.bfloat16, mybir.dt.uint16
    i32, i16 = mybir.dt.int32, mybir.dt.int16

    nc = bacc.Bacc("TRN2", target_bir_lowering=False, debug=False,
                   num_devices=NCORES)

    has_att = stage > 0
    has_tf = stage < 3
    att_l = stage - 1  # layer index of the attention part (0,1,2)

    # ---- external IO ----
    if stage == 0:
        xT_d = nc.dram_tensor("xT", [IN_CH, NPC], bf16, kind="ExternalInput")
        W_d = nc.dram_tensor("W", [IN_CH, CH], bf16, kind="ExternalInput")
    elif has_tf:
        W_d = nc.dram_tensor("W", [CH, CH], bf16, kind="ExternalInput")
    if has_tf:
        avs_d = nc.dram_tensor("avs", [128, CH], f32, kind="ExternalInput")
        avd_d = nc.dram_tensor("avd", [128, CH], f32, kind="ExternalInput")
        tloc_o = nc.dram_tensor("tloc_o", [NPCP, ROW_U16], u16, kind="ExternalOutput")
        adst_o = nc.dram_tensor("adst_o", [T * 128, 1], f32, kind="ExternalOutput")
    if has_att:
        tfull_d = nc.dram_tensor("tfull", [NCORES * NPCP, ROW_U16], u16,
                                 kind="ExternalInput")
        adst_d = nc.dram_tensor("adst", [T * 128, 1], f32, kind="ExternalInput")
        srcoff_d = nc.dram_tensor("srcoff", [128, G], i32, kind="ExternalInput")
        adstoff_d = nc.dram_tensor("adstoff", [128, G], i32, kind="ExternalInput")
        stidx_d = nc.dram_tensor("stidx", [128, G], i16, kind="ExternalInput")
        bias_d = nc.dram_tensor("bias", [128, CH], f32, kind="ExternalInput")
    if stage == 2:
        mean_o = nc.dram_tensor("mean_o", [NPC, CH], f32, kind="ExternalOutput")
        alpha_o = nc.dram_tensor("alpha_o", [128, G], f32, kind="ExternalOutput")
    if stage == 3:
        var_o = nc.dram_tensor("var_o", [NPC, CH], f32, kind="ExternalOutput")

    mult = mybir.AluOpType.mult
    add = mybir.AluOpType.add
    amax = mybir.AluOpType.max
    Copy = mybir.ActivationFunctionType.Copy
    Exp = mybir.ActivationFunctionType.Exp

    with tile.TileContext(nc) as tc:
        with (
            tc.tile_pool(name="constp", bufs=1) as constp,
            tc.tile_pool(name="pers", bufs=1) as pers,
            tc.tile_pool(name="work", bufs=4) as work,
            tc.tile_pool(name="gatp", bufs=3) as gatp,
            tc.tile_pool(name="stp", bufs=3) as stp,
            tc.tile_pool(name="psA", bufs=4, space="PSUM") as psA,
            tc.tile_pool(name="psB", bufs=2, space="PSUM") as psB,
            tc.tile_pool(name="dramp", bufs=1, space="DRAM") as dramp,
        ):
            def load(name, d_ap, shape, dtype):
                t = constp.tile(shape, dtype, name=name)
                nc.sync.dma_start(out=t[:], in_=d_ap)
                return t

            # ---- constants ----
            if has_att:
                ident = constp.tile([128, 128], bf16, name="ident")
                make_identity(nc, ident[:])
                srcoff_sb = load("srcoffs", srcoff_d[:], [128, G], i32)
                adstoff_sb = load("adstoffs", adstoff_d[:], [128, G], i32)
                stidx_sb = load("stidxs", stidx_d[:], [128, G], i16)
                bias_sb = load("biass", bias_d[:], [128, CH], f32)

            if has_tf:
                avs_sb = load("avss", avs_d[:], [128, CH], f32)
                avd_sb = load("avds", avd_d[:], [128, CH], f32)
                asrc_all = pers.tile([128, T], f32, name="asrca")
                adst_node = pers.tile([128, T], f32, name="adstn")
                if stage == 0:
                    W_sb = [load(f"Ws{k}", W_d[k * 128:(k + 1) * 128, :], [128, CH], bf16)
                            for k in range(IN_CH // 128)]
                    xT_sb = []
                    for k in range(IN_CH // 128):
                        xt = constp.tile([128, NPCP], bf16, name=f"xT_{k}")
                        if NPCP > NPC:
                            nc.vector.memset(xt[:, NPC:NPCP], 0.0)
                        nc.sync.dma_start(out=xt[:, 0:NPC],
                                          in_=xT_d[k * 128:(k + 1) * 128, :])
                        xT_sb.append(xt)
                else:
                    W_sb = [load("Ws", W_d[:], [CH, CH], bf16)]

            if has_att:
                hT_sb = pers.tile([128, NPCP], bf16, name="hT")  # att output ^T
                r_all = pers.tile([128, T], f32, name="rall")
                if stage == 2:
                    exp_all = pers.tile([128, G], f32, name="expall")
                    rden_dram = dramp.tile([T * 128, 1], f32, name="rdend")

            # ------------------------------------------------------------------
            def transform(lhsT_of):
                for t in range(T):
                    n0 = t * 128
                    tp = psB.tile([128, CH], f32, name="tf_ps")
                    nk = len(W_sb)
                    for ki in range(nk):
                        nc.tensor.matmul(
                            out=tp[:], lhsT=lhsT_of(ki, n0), rhs=W_sb[ki][:],
                            start=(ki == 0), stop=(ki == nk - 1),
                        )
                    tta = work.tile([128, CH], f32, name="tta")
                    nc.vector.tensor_tensor(out=tta[:], in0=tp[:], in1=avs_sb[:],
                                            op=mult)
                    nc.vector.tensor_reduce(
                        out=asrc_all[:, t:t + 1], in_=tta[:],
                        axis=mybir.AxisListType.X, op=add,
                    )
                    ttb = work.tile([128, CH], f32, name="ttb")
                    nc.vector.tensor_tensor(out=ttb[:], in0=tp[:], in1=avd_sb[:],
                                            op=mult)
                    nc.vector.tensor_reduce(
                        out=adst_node[:, t:t + 1], in_=ttb[:],
                        axis=mybir.AxisListType.X, op=add,
                    )
                    stg = work.tile([128, ROW_U16], u16, name="stg")
                    stgb = stg[:].bitcast(bf16)
                    nc.scalar.activation(out=stgb[:, 0:128], in_=tp[:], func=Copy)
                    nc.vector.memset(stgb[:, 128:129], 1.0)
                    nc.vector.memset(stgb[:, 129:130], 0.0)
                    nc.vector.tensor_copy(
                        out=stg[:].bitcast(f32)[:, 65:66],
                        in_=asrc_all[:, t:t + 1],
                    )
                    nc.sync.dma_start(out=tloc_o[n0:n0 + 128, :], in_=stg[:])
                nc.sync.dma_start(out=adst_o[:], in_=adst_node[:])

            # ------------------------------------------------------------------
            def attention():
                first_of = {}
                last_of = {}
                for g, t in enumerate(tmap):
                    t = int(t)
                    if t not in first_of:
                        first_of[t] = g
                    last_of[t] = g

                adall = work.tile([128, G], f32, name="adall", bufs=1)

                open_ps = {}

                def epilogue(t, ps):
                    rows = min(128, NPC - t * 128)
                    dm = work.tile([128, 1], f32, name="dm")
                    nc.vector.tensor_scalar_max(out=dm[:], in0=ps[:, 128:129],
                                                scalar1=1e-16)
                    nc.vector.reciprocal(out=r_all[:, t:t + 1], in_=dm[:])
                    osb = work.tile([128, CH], f32, name="osb")
                    nc.scalar.activation(out=osb[:], in_=ps[:, 0:128], func=Copy,
                                         scale=r_all[:, t:t + 1])
                    nc.vector.tensor_tensor(
                        out=osb[:], in0=osb[:], in1=bias_sb[:], op=add,
                    )
                    if stage == 2:
                        nc.sync.dma_start(out=mean_o[t * 128:t * 128 + rows, :],
                                          in_=osb[0:rows, :])
                    if stage == 3:
                        nc.sync.dma_start(out=var_o[t * 128:t * 128 + rows, :],
                                          in_=osb[0:rows, :])
                        return
                    hb = work.tile([128, CH], bf16, name="hb")
                    if stage == 1:  # relu for h2
                        nc.vector.tensor_scalar_max(out=hb[:], in0=osb[:], scalar1=0.0)
                    else:
                        nc.vector.tensor_copy(out=hb[:], in_=osb[:])
                    tps = psB.tile([128, CH], bf16, name="tr_ps")
                    nc.tensor.transpose(out=tps[:], in_=hb[:], identity=ident[:])
                    nc.scalar.activation(out=hT_sb[:, t * 128:(t + 1) * 128],
                                         in_=tps[:], func=Copy)

                for gg0 in range(0, G, GG):
                    gg1 = min(G, gg0 + GG)
                    gn = gg1 - gg0
                    gt = gatp.tile([128, GG, ROW_U16], u16, name="gt")
                    for c in range(gn):
                        nc.gpsimd.indirect_dma_start(
                            out=gt[:, c, :], out_offset=None,
                            in_=tfull_d[:],
                            in_offset=bass.IndirectOffsetOnAxis(
                                ap=srcoff_sb[:, gg0 + c:gg0 + c + 1], axis=0),
                        )
                        nc.gpsimd.indirect_dma_start(
                            out=adall[:, gg0 + c:gg0 + c + 1], out_offset=None,
                            in_=adst_d[:],
                            in_offset=bass.IndirectOffsetOnAxis(
                                ap=adstoff_sb[:, gg0 + c:gg0 + c + 1], axis=0),
                        )
                    gf32 = gt[:].bitcast(f32)
                    z = work.tile([128, GG], f32, name="z")
                    nc.vector.tensor_tensor(out=z[:, 0:gn], in0=gf32[:, 0:gn, 65],
                                            in1=adall[:, gg0:gg1], op=add)
                    zs = work.tile([128, GG], f32, name="zs")
                    nc.vector.tensor_scalar_mul(out=zs[:, 0:gn], in0=z[:, 0:gn],
                                                scalar1=NEG_SLOPE)
                    nc.vector.tensor_tensor(out=z[:, 0:gn], in0=z[:, 0:gn],
                                            in1=zs[:, 0:gn], op=amax)
                    if stage == 2:
                        ef = exp_all[:, gg0:gg1]
                    else:
                        eft = work.tile([128, GG], f32, name="eft")
                        ef = eft[:, 0:gn]
                    nc.scalar.activation(out=ef, in_=z[:, 0:gn], func=Exp)
                    eb = work.tile([128, GG], bf16, name="eb")
                    nc.vector.tensor_copy(out=eb[:, 0:gn], in_=ef)

                    for ls0 in range(gg0, gg1, LS):
                        st = stp.tile([128, LS * 128], bf16, name="st")
                        nc.gpsimd.local_scatter(
                            out_ap=st[:],
                            data_ap=eb[:, ls0 - gg0:ls0 - gg0 + LS],
                            idxs_ap=stidx_sb[:, ls0:ls0 + LS],
                            channels=128, num_elems=LS * 128, num_idxs=LS,
                        )
                        for j in range(LS):
                            g = ls0 + j
                            t = int(tmap[g])
                            first = first_of[t] == g
                            last = last_of[t] == g
                            if first:
                                open_ps[t] = psA.tile([128, CH + 1], f32,
                                                      name="att_ps")
                            ps = open_ps.pop(t) if last else open_ps[t]
                            nc.tensor.matmul(
                                out=ps[:],
                                lhsT=st[:, j * 128:(j + 1) * 128],
                                rhs=gt[:, g - gg0, 0:129].bitcast(bf16),
                                start=first, stop=last,
                            )
                            if last:
                                epilogue(t, ps)

                if stage == 2:
                    nc.sync.dma_start(out=rden_dram[:], in_=r_all[:])
                    rg = pers.tile([128, G], f32, name="rg")
                    for g in range(G):
                        nc.gpsimd.indirect_dma_start(
                            out=rg[:, g:g + 1], out_offset=None,
                            in_=rden_dram[:],
                            in_offset=bass.IndirectOffsetOnAxis(
                                ap=adstoff_sb[:, g:g + 1], axis=0),
                        )
                    av = pers.tile([128, G], f32, name="av")
                    nc.vector.tensor_tensor(out=av[:], in0=exp_all[:], in1=rg[:],
                                            op=mult)
                    nc.sync.dma_start(out=alpha_o[:], in_=av[:])

            # ------------------------------------------------------------------
            if has_att:
                attention()
            if has_tf:
                if stage == 0:
                    transform(lambda ki, n0: xT_sb[ki][:, n0:n0 + 128])
                else:
                    transform(lambda ki, n0: hT_sb[:, n0:n0 + 128])

    nc.compile()
    return nc


# --------------------------------------------------------------------------
# Entry point
# --------------------------------------------------------------------------

def _prepare(inputs):
    NPC = _npc()
    x = np.asarray(inputs["x"], dtype=np.float32)
    ei = np.asarray(inputs["edge_index"], dtype=np.int64)
    src, dst = ei[0], ei[1]
    plan12 = _plan_layer(src, dst)
    loop = np.arange(N_NODES, dtype=np.int64)
    plan3 = _plan_layer(np.concatenate([src, loop]), np.concatenate([dst, loop]))

    def as_bf(a):
        return np.asarray(a, dtype=np.float32).astype(BF16)

    def rep(a):
        return np.ascontiguousarray(
            np.broadcast_to(np.asarray(a, np.float32).reshape(1, CH), (128, CH))
        )

    per_core = []
    for c in range(NCORES):
        p12 = plan12["cores"][c]
        p3 = plan3["cores"][c]
        per_core.append(dict(
            xT=np.ascontiguousarray(as_bf(x[c * NPC:(c + 1) * NPC]).T),
            W1=as_bf(inputs["W1"]), W2=as_bf(inputs["W2"]), Wv=as_bf(inputs["Wv"]),
            as1=rep(inputs["as1"]), ad1=rep(inputs["ad1"]),
            as2=rep(inputs["as2"]), ad2=rep(inputs["ad2"]),
            asv=rep(inputs["asv"]), adv=rep(inputs["adv"]),
            b1=rep(inputs["b1"]), b2=rep(inputs["b2"]), bv=rep(inputs["bv"]),
            srcoff12=p12["src_off"], adstoff12=p12["adst_off"], stidx12=p12["st_idx"],
            srcoff3=p3["src_off"], adstoff3=p3["adst_off"], stidx3=p3["st_idx"],
        ))
    return plan12, plan3, per_core


def _stage_inputs(stage, pc, tfull, adst):
    if stage == 0:
        return {"xT": pc["xT"], "W": pc["W1"], "avs": pc["as1"], "avd": pc["ad1"]}
    if stage == 1:
        return {"tfull": tfull, "adst": adst, "srcoff": pc["srcoff12"],
                "adstoff": pc["adstoff12"], "stidx": pc["stidx12"],
                "bias": pc["b1"], "W": pc["W2"], "avs": pc["as2"], "avd": pc["ad2"]}
    if stage == 2:
        return {"tfull": tfull, "adst": adst, "srcoff": pc["srcoff12"],
                "adstoff": pc["adstoff12"], "stidx": pc["stidx12"],
                "bias": pc["b2"], "W": pc["Wv"], "avs": pc["asv"], "avd": pc["adv"]}
    return {"tfull": tfull, "adst": adst, "srcoff": pc["srcoff3"],
            "adstoff": pc["adstoff3"], "stidx": pc["stidx3"], "bias": pc["bv"]}


_CACHED = {}


def _get_programs(G12, G3, tmap12, tmap3):
    key = (G12, G3)
    if key not in _CACHED:
        progs = []
        for stage in range(4):
            G, tm = (G3, tmap3) if stage == 3 else (G12, tmap12)
            progs.append(_build_stage(stage, G, tm))
        _CACHED[key] = progs
    return _CACHED[key]


def run_pipeline(plan12, plan3, per_core, runner):
    """runner(nc, in_maps) -> (results list, exec_ns or None)"""
    progs = _get_programs(plan12["G"], plan3["G"],
                          plan12["tile_of_chunk"], plan3["tile_of_chunk"])
    tfull = None
    adst = [None] * NCORES
    outs = {}
    total_ns = 0
    have_ns = True
    for stage in range(4):
        in_maps = [_stage_inputs(stage, per_core[c], tfull, adst[c])
                   for c in range(NCORES)]
        results, ns = runner(progs[stage], in_maps)
        if ns is None:
            have_ns = False
        else:
            total_ns += ns
        if stage < 3:
            tfull = np.concatenate([results[c]["tloc_o"] for c in range(NCORES)],
                                   axis=0)
            adst = [results[c]["adst_o"] for c in range(NCORES)]
        if stage == 2:
            outs["mean"] = [results[c]["mean_o"] for c in range(NCORES)]
            outs["alpha"] = [results[c]["alpha_o"] for c in range(NCORES)]
        if stage == 3:
            outs["var"] = [results[c]["var_o"] for c in range(NCORES)]
    return outs, (total_ns if have_ns else None)


def _assemble(plan12, outs):
    NPC = _npc()
    mean = np.empty((N_NODES, CH), dtype=np.float32)
    var = np.empty((N_NODES, CH), dtype=np.float32)
    alpha = np.empty(N_EDGES, dtype=np.float32)
    for c in range(NCORES):
        mean[c * NPC:(c + 1) * NPC] = outs["mean"][c]
        var[c * NPC:(c + 1) * NPC] = outs["var"][c]
        pc = plan12["cores"][c]
        alpha[pc["orig"]] = outs["alpha"][c][pc["slot_p"], pc["slot_g"]]
    return mean, var, alpha


def _hw_runner(nc, in_maps, trace=False):
    from concourse.bass_utils import run_bass_kernel_spmd

    res = run_bass_kernel_spmd(nc, in_maps, list(range(NCORES)), trace=trace)
    return res.results, res.exec_time_ns


def kernel(**inputs):
    plan12, plan3, per_core = _prepare(inputs)
    outs, _ = run_pipeline(plan12, plan3, per_core, _hw_runner)
    return _assemble(plan12, outs)


# revision 25
# speedup vs baseline: 2.0676x; 2.0676x over previous
"""3-layer GAT (negbin head) Trainium2 Bass kernel, 8-core SPMD.

Strategy (graph/data parallel, dst-sharded):
  - Nodes are sharded contiguously across 8 cores (dst ownership).
  - Per layer: each core transforms its own nodes (t = h @ W, node-major
    PE matmuls) and packs a gather-table row [h_bf16(128) | 1.0 | pad |
    a_src_f32] per node. Tables are exchanged between launches (host
    concat; collectives are unavailable through this NRT path).
  - Attention: per-edge rows are fetched with indirect DMA (one 264B
    descriptor per edge), logits z = a_src[src]+a_dst[dst] -> leaky_relu ->
    exp on DVE/ACT, exp values scattered into one-hot "St" matrices on
    GpSimd (local_scatter), and one PE matmul per 128-edge chunk computes
    both the exp-weighted feature sum and the softmax denominator (via the
    table's ones column) into PSUM. A per-tile epilogue normalizes, adds
    bias, and prepares the next layer's transform input in SBUF.
  - Edge bookkeeping (sorting by dst, chunking, padding so all cores run
    the identical SPMD program) is integer-only host preprocessing.

Four launches: [tf1] -> [att1+tf2] -> [att2+tf3] -> [att3]; the attention
layer feeding a transform stays in one launch (h2T/meanT live in SBUF).
"""
import sys

sys.path.insert(0, "/opt/trn_rl_repo")

import numpy as np
import ml_dtypes

from concourse import bass, bacc, mybir, tile
from concourse.masks import make_identity

BF16 = ml_dtypes.bfloat16

# ---- problem constants (hardcoded; harness provides full-size inputs) ----
N_NODES = 50000
N_EDGES = 600000
IN_CH = 512
CH = 128
NCORES = 8
NEG_SLOPE = 0.2

ROW_U16 = 132  # u16 slots per row: [0:128]=h bf16, 128:1.0, 129:pad, 130:132=a_src f32
LS = 14        # chunks per local_scatter group (num_elems = 14*128 = 1792 <= 2046)
GG = 42        # chunks per gather group (= 3 local_scatter groups)


def _npc():
    return N_NODES // NCORES


def _t_tiles():
    return (_npc() + 127) // 128


def _npc_pad():
    return _t_tiles() * 128


# --------------------------------------------------------------------------
# Host planning: integer-only edge bookkeeping
# --------------------------------------------------------------------------

def _plan_layer(src, dst):
    """src/dst: global node id arrays. Returns per-core slot arrays + shared
    chunk->tile map. All cores get the same G and tile map (SPMD)."""
    NPC, T, NPCP = _npc(), _t_tiles(), _npc_pad()
    per_core = []
    tile_counts = []
    k = np.zeros(T, dtype=np.int64)
    for c in range(NCORES):
        lo, hi = c * NPC, (c + 1) * NPC
        idx = np.nonzero((dst >= lo) & (dst < hi))[0]
        es = src[idx].astype(np.int64)
        ed = (dst[idx] - lo).astype(np.int64)
        order = np.argsort(ed, kind="stable")
        es, ed, idx = es[order], ed[order], idx[order]
        per_core.append((es, ed, idx))
        cnt = np.bincount(ed // 128, minlength=T)
        tile_counts.append(cnt)
        k = np.maximum(k, (cnt + 127) // 128)
    k = np.maximum(k, 1)
    G = int(k.sum())
    G_pad = ((G + LS - 1) // LS) * LS
    k[-1] += G_pad - G
    G = G_pad
    tile_of_chunk = np.repeat(np.arange(T), k)
    chunk_start = np.zeros(T, dtype=np.int64)
    chunk_start[1:] = np.cumsum(k)[:-1]

    cores = []
    for c in range(NCORES):
        es, ed, orig = per_core[c]
        src_off = np.zeros((128, G), dtype=np.int32)
        adst_off = np.zeros((128, G), dtype=np.int32)
        st_idx = np.full((128, G), -1, dtype=np.int16)
        slot_p = np.zeros(len(es), dtype=np.int64)
        slot_g = np.zeros(len(es), dtype=np.int64)
        tstarts = np.zeros(T + 1, dtype=np.int64)
        tstarts[1:] = np.cumsum(tile_counts[c])
        for t in range(T):
            e0, e1 = int(tstarts[t]), int(tstarts[t + 1])
            n = e1 - e0
            if n == 0:
                continue
            i = np.arange(n)
            g = chunk_start[t] + i // 128
            p = i % 128
            gsrc = es[e0:e1]
            src_off[p, g] = ((gsrc // NPC) * NPCP + (gsrc % NPC)).astype(np.int32)
            dl = ed[e0:e1]
            adst_off[p, g] = dl.astype(np.int32)  # t-major tables: row = dst local
            st_idx[p, g] = ((g % LS) * 128 + (dl % 128)).astype(np.int16)
            slot_p[e0:e1] = p
            slot_g[e0:e1] = g
        cores.append(
            dict(src_off=src_off, adst_off=adst_off, st_idx=st_idx,
                 orig=orig, slot_p=slot_p, slot_g=slot_g)
        )
    return dict(G=G, tile_of_chunk=tile_of_chunk, k=k, cores=cores)


# --------------------------------------------------------------------------
# Stage program builders
# --------------------------------------------------------------------------
# stage 0: tf(L1)            ins: xT,W1,as1,ad1            outs: tloc,adst
# stage 1: att(L1)+tf(L2)    ins: tfull,adst,offs12,b1,W2,as2,ad2   outs: tloc,adst
# stage 2: att(L2)+tf(L3)    ins: tfull,adst,offs12,b2,Wv,asv,adv   outs: mean,alpha,tloc,adst
# stage 3: att(L3)           ins: tfull,adst,offs3,bv      outs: var

def _build_stage(stage, G, tmap):
    NPC, T, NPCP = _npc(), _t_tiles(), _npc_pad()
    f32, bf16, u16 = mybir.dt.float32, mybir.dt.bfloat16, mybir.dt.uint16
    i32, i16 = mybir.dt.int32, mybir.dt.int16

    nc = bacc.Bacc("TRN2", target_bir_lowering=False, debug=False,
                   num_devices=NCORES)

    has_att = stage > 0
    has_tf = stage < 3
    att_l = stage - 1  # layer index of the attention part (0,1,2)

    # ---- external IO ----
    if stage == 0:
        xT_d = nc.dram_tensor("xT", [IN_CH, NPC], bf16, kind="ExternalInput")
        W_d = nc.dram_tensor("W", [IN_CH, CH], bf16, kind="ExternalInput")
    elif has_tf:
        W_d = nc.dram_tensor("W", [CH, CH], bf16, kind="ExternalInput")
    if has_tf:
        avs_d = nc.dram_tensor("avs", [128, CH], f32, kind="ExternalInput")
        avd_d = nc.dram_tensor("avd", [128, CH], f32, kind="ExternalInput")
        tloc_o = nc.dram_tensor("tloc_o", [NPCP, ROW_U16], u16, kind="ExternalOutput")
        adst_o = nc.dram_tensor("adst_o", [T * 128, 1], f32, kind="ExternalOutput")
    if has_att:
        tfull_d = nc.dram_tensor("tfull", [NCORES * NPCP, ROW_U16], u16,
                                 kind="ExternalInput")
        adst_d = nc.dram_tensor("adst", [T * 128, 1], f32, kind="ExternalInput")
        srcoff_d = nc.dram_tensor("srcoff", [128, G], i32, kind="ExternalInput")
        adstoff_d = nc.dram_tensor("adstoff", [128, G], i32, kind="ExternalInput")
        stidx_d = nc.dram_tensor("stidx", [128, G], i16, kind="ExternalInput")
        bias_d = nc.dram_tensor("bias", [128, CH], f32, kind="ExternalInput")
    if stage == 2:
        mean_o = nc.dram_tensor("mean_o", [NPC, CH], f32, kind="ExternalOutput")
        alpha_o = nc.dram_tensor("alpha_o", [128, G], f32, kind="ExternalOutput")
    if stage == 3:
        var_o = nc.dram_tensor("var_o", [NPC, CH], f32, kind="ExternalOutput")

    mult = mybir.AluOpType.mult
    add = mybir.AluOpType.add
    amax = mybir.AluOpType.max
    Copy = mybir.ActivationFunctionType.Copy
    Exp = mybir.ActivationFunctionType.Exp

    with tile.TileContext(nc) as tc:
        with (
            tc.tile_pool(name="constp", bufs=1) as constp,
            tc.tile_pool(name="pers", bufs=1) as pers,
            tc.tile_pool(name="work", bufs=4) as work,
            tc.tile_pool(name="gatp", bufs=3) as gatp,
            tc.tile_pool(name="stp", bufs=3) as stp,
            tc.tile_pool(name="psA", bufs=4, space="PSUM") as psA,
            tc.tile_pool(name="psB", bufs=2, space="PSUM") as psB,
            tc.tile_pool(name="dramp", bufs=1, space="DRAM") as dramp,
        ):
            def load(name, d_ap, shape, dtype):
                t = constp.tile(shape, dtype, name=name)
                nc.sync.dma_start(out=t[:], in_=d_ap)
                return t

            # ---- constants ----
            if has_att:
                ident = constp.tile([128, 128], bf16, name="ident")
                make_identity(nc, ident[:])
                srcoff_sb = load("srcoffs", srcoff_d[:], [128, G], i32)
                adstoff_sb = load("adstoffs", adstoff_d[:], [128, G], i32)
                stidx_sb = load("stidxs", stidx_d[:], [128, G], i16)
                bias_sb = load("biass", bias_d[:], [128, CH], f32)

            if has_tf:
                avs_sb = load("avss", avs_d[:], [128, CH], f32)
                avd_sb = load("avds", avd_d[:], [128, CH], f32)
                asrc_all = pers.tile([128, T], f32, name="asrca")
                adst_node = pers.tile([128, T], f32, name="adstn")
                if stage == 0:
                    W_sb = [load(f"Ws{k}", W_d[k * 128:(k + 1) * 128, :], [128, CH], bf16)
                            for k in range(IN_CH // 128)]
                    xT_sb = []
                    for k in range(IN_CH // 128):
                        xt = constp.tile([128, NPCP], bf16, name=f"xT_{k}")
                        if NPCP > NPC:
                            nc.vector.memset(xt[:, NPC:NPCP], 0.0)
                        nc.sync.dma_start(out=xt[:, 0:NPC],
                                          in_=xT_d[k * 128:(k + 1) * 128, :])
                        xT_sb.append(xt)
                else:
                    W_sb = [load("Ws", W_d[:], [CH, CH], bf16)]

            if has_att:
                hT_sb = pers.tile([128, NPCP], bf16, name="hT")  # att output ^T
                r_all = pers.tile([128, T], f32, name="rall")
                if stage == 2:
                    exp_all = pers.tile([128, G], f32, name="expall")
                    rden_dram = dramp.tile([T * 128, 1], f32, name="rdend")

            # ------------------------------------------------------------------
            def transform(lhsT_of):
                for t in range(T):
                    n0 = t * 128
                    tp = psB.tile([128, CH], f32, name="tf_ps")
                    nk = len(W_sb)
                    for ki in range(nk):
                        nc.tensor.matmul(
                            out=tp[:], lhsT=lhsT_of(ki, n0), rhs=W_sb[ki][:],
                            start=(ki == 0), stop=(ki == nk - 1),
                        )
                    tta = work.tile([128, CH], f32, name="tta")
                    nc.vector.tensor_tensor(out=tta[:], in0=tp[:], in1=avs_sb[:],
                                            op=mult)
                    nc.vector.tensor_reduce(
                        out=asrc_all[:, t:t + 1], in_=tta[:],
                        axis=mybir.AxisListType.X, op=add,
                    )
                    ttb = work.tile([128, CH], f32, name="ttb")
                    nc.vector.tensor_tensor(out=ttb[:], in0=tp[:], in1=avd_sb[:],
                                            op=mult)
                    nc.vector.tensor_reduce(
                        out=adst_node[:, t:t + 1], in_=ttb[:],
                        axis=mybir.AxisListType.X, op=add,
                    )
                    stg = work.tile([128, ROW_U16], u16, name="stg")
                    stgb = stg[:].bitcast(bf16)
                    nc.scalar.activation(out=stgb[:, 0:128], in_=tp[:], func=Copy)
                    nc.vector.memset(stgb[:, 128:129], 1.0)
                    nc.vector.memset(stgb[:, 129:130], 0.0)
                    nc.vector.tensor_copy(
                        out=stg[:].bitcast(f32)[:, 65:66],
                        in_=asrc_all[:, t:t + 1],
                    )
                    nc.sync.dma_start(out=tloc_o[n0:n0 + 128, :], in_=stg[:])
                    nc.sync.dma_start(out=adst_o[n0:n0 + 128, :],
                                      in_=adst_node[:, t:t + 1])

            # ------------------------------------------------------------------
            def attention():
                first_of = {}
                last_of = {}
                for g, t in enumerate(tmap):
                    t = int(t)
                    if t not in first_of:
                        first_of[t] = g
                    last_of[t] = g

                adall = work.tile([128, G], f32, name="adall", bufs=1)
                if stage == 2:
                    rg = pers.tile([128, G], f32, name="rg")
                    zr = work.tile([128, T], f32, name="zr")
                    nc.vector.memset(zr[:], 0.0)
                    nc.sync.dma_start(out=rden_dram[:], in_=zr[:])

                open_ps = {}

                def epilogue(t, ps):
                    rows = min(128, NPC - t * 128)
                    dm = work.tile([128, 1], f32, name="dm")
                    nc.vector.tensor_scalar_max(out=dm[:], in0=ps[:, 128:129],
                                                scalar1=1e-16)
                    nc.vector.reciprocal(out=r_all[:, t:t + 1], in_=dm[:])
                    osb = work.tile([128, CH], f32, name="osb")
                    nc.scalar.activation(out=osb[:], in_=ps[:, 0:128], func=Copy,
                                         scale=r_all[:, t:t + 1])
                    nc.vector.tensor_tensor(
                        out=osb[:], in0=osb[:], in1=bias_sb[:], op=add,
                    )
                    if stage == 2:
                        nc.sync.dma_start(out=mean_o[t * 128:t * 128 + rows, :],
                                          in_=osb[0:rows, :])
                        # write this tile's reciprocal rows (t-major) and
                        # gather the per-edge denominator for its chunks now,
                        # so the alpha tail overlaps the remaining sweep.
                        nc.sync.dma_start(
                            out=rden_dram[t * 128:(t + 1) * 128, :],
                            in_=r_all[:, t:t + 1])
                        for g in range(first_of[t], last_of[t] + 1):
                            nc.gpsimd.indirect_dma_start(
                                out=rg[:, g:g + 1], out_offset=None,
                                in_=rden_dram[:],
                                in_offset=bass.IndirectOffsetOnAxis(
                                    ap=adstoff_sb[:, g:g + 1], axis=0),
                            )
                    if stage == 3:
                        nc.sync.dma_start(out=var_o[t * 128:t * 128 + rows, :],
                                          in_=osb[0:rows, :])
                        return
                    hb = work.tile([128, CH], bf16, name="hb")
                    if stage == 1:  # relu for h2
                        nc.vector.tensor_scalar_max(out=hb[:], in0=osb[:], scalar1=0.0)
                    else:
                        nc.vector.tensor_copy(out=hb[:], in_=osb[:])
                    tps = psB.tile([128, CH], bf16, name="tr_ps")
                    nc.tensor.transpose(out=tps[:], in_=hb[:], identity=ident[:])
                    nc.scalar.activation(out=hT_sb[:, t * 128:(t + 1) * 128],
                                         in_=tps[:], func=Copy)

                for gg0 in range(0, G, GG):
                    gg1 = min(G, gg0 + GG)
                    gn = gg1 - gg0
                    gt = gatp.tile([128, GG, ROW_U16], u16, name="gt")
                    for c in range(gn):
                        nc.gpsimd.indirect_dma_start(
                            out=gt[:, c, :], out_offset=None,
                            in_=tfull_d[:],
                            in_offset=bass.IndirectOffsetOnAxis(
                                ap=srcoff_sb[:, gg0 + c:gg0 + c + 1], axis=0),
                        )
                        nc.gpsimd.indirect_dma_start(
                            out=adall[:, gg0 + c:gg0 + c + 1], out_offset=None,
                            in_=adst_d[:],
                            in_offset=bass.IndirectOffsetOnAxis(
                                ap=adstoff_sb[:, gg0 + c:gg0 + c + 1], axis=0),
                        )
                    gf32 = gt[:].bitcast(f32)
                    z = work.tile([128, GG], f32, name="z")
                    nc.vector.tensor_tensor(out=z[:, 0:gn], in0=gf32[:, 0:gn, 65],
                                            in1=adall[:, gg0:gg1], op=add)
                    zs = work.tile([128, GG], f32, name="zs")
                    nc.vector.tensor_scalar_mul(out=zs[:, 0:gn], in0=z[:, 0:gn],
                                                scalar1=NEG_SLOPE)
                    nc.vector.tensor_tensor(out=z[:, 0:gn], in0=z[:, 0:gn],
                                            in1=zs[:, 0:gn], op=amax)
                    if stage == 2:
                        ef = exp_all[:, gg0:gg1]
                    else:
                        eft = work.tile([128, GG], f32, name="eft")
                        ef = eft[:, 0:gn]
                    nc.scalar.activation(out=ef, in_=z[:, 0:gn], func=Exp)
                    eb = work.tile([128, GG], bf16, name="eb")
                    nc.vector.tensor_copy(out=eb[:, 0:gn], in_=ef)

                    for ls0 in range(gg0, gg1, LS):
                        st = stp.tile([128, LS * 128], bf16, name="st")
                        nc.gpsimd.local_scatter(
                            out_ap=st[:],
                            data_ap=eb[:, ls0 - gg0:ls0 - gg0 + LS],
                            idxs_ap=stidx_sb[:, ls0:ls0 + LS],
                            channels=128, num_elems=LS * 128, num_idxs=LS,
                        )
                        for j in range(LS):
                            g = ls0 + j
                            t = int(tmap[g])
                            first = first_of[t] == g
                            last = last_of[t] == g
                            if first:
                                open_ps[t] = psA.tile([128, CH + 1], f32,
                                                      name="att_ps")
                            ps = open_ps.pop(t) if last else open_ps[t]
                            nc.tensor.matmul(
                                out=ps[:],
                                lhsT=st[:, j * 128:(j + 1) * 128],
                                rhs=gt[:, g - gg0, 0:129].bitcast(bf16),
                                start=first, stop=last,
                            )
                            if last:
                                epilogue(t, ps)

                if stage == 2:
                    av = pers.tile([128, G], f32, name="av")
                    nc.vector.tensor_tensor(out=av[:], in0=exp_all[:], in1=rg[:],
                                            op=mult)
                    nc.sync.dma_start(out=alpha_o[:], in_=av[:])

            # ------------------------------------------------------------------
            if has_att:
                attention()
            if has_tf:
                if stage == 0:
                    transform(lambda ki, n0: xT_sb[ki][:, n0:n0 + 128])
                else:
                    transform(lambda ki, n0: hT_sb[:, n0:n0 + 128])

    nc.compile()
    return nc


# --------------------------------------------------------------------------
# Entry point
# --------------------------------------------------------------------------

def _prepare(inputs):
    NPC = _npc()
    x = np.asarray(inputs["x"], dtype=np.float32)
    ei = np.asarray(inputs["edge_index"], dtype=np.int64)
    src, dst = ei[0], ei[1]
    plan12 = _plan_layer(src, dst)
    loop = np.arange(N_NODES, dtype=np.int64)
    plan3 = _plan_layer(np.concatenate([src, loop]), np.concatenate([dst, loop]))

    def as_bf(a):
        return np.asarray(a, dtype=np.float32).astype(BF16)

    def rep(a):
        return np.ascontiguousarray(
            np.broadcast_to(np.asarray(a, np.float32).reshape(1, CH), (128, CH))
        )

    per_core = []
    for c in range(NCORES):
        p12 = plan12["cores"][c]
        p3 = plan3["cores"][c]
        per_core.append(dict(
            xT=np.ascontiguousarray(as_bf(x[c * NPC:(c + 1) * NPC]).T),
            W1=as_bf(inputs["W1"]), W2=as_bf(inputs["W2"]), Wv=as_bf(inputs["Wv"]),
            as1=rep(inputs["as1"]), ad1=rep(inputs["ad1"]),
            as2=rep(inputs["as2"]), ad2=rep(inputs["ad2"]),
            asv=rep(inputs["asv"]), adv=rep(inputs["adv"]),
            b1=rep(inputs["b1"]), b2=rep(inputs["b2"]), bv=rep(inputs["bv"]),
            srcoff12=p12["src_off"], adstoff12=p12["adst_off"], stidx12=p12["st_idx"],
            srcoff3=p3["src_off"], adstoff3=p3["adst_off"], stidx3=p3["st_idx"],
        ))
    return plan12, plan3, per_core


def _stage_inputs(stage, pc, tfull, adst):
    if stage == 0:
        return {"xT": pc["xT"], "W": pc["W1"], "avs": pc["as1"], "avd": pc["ad1"]}
    if stage == 1:
        return {"tfull": tfull, "adst": adst, "srcoff": pc["srcoff12"],
                "adstoff": pc["adstoff12"], "stidx": pc["stidx12"],
                "bias": pc["b1"], "W": pc["W2"], "avs": pc["as2"], "avd": pc["ad2"]}
    if stage == 2:
        return {"tfull": tfull, "adst": adst, "srcoff": pc["srcoff12"],
                "adstoff": pc["adstoff12"], "stidx": pc["stidx12"],
                "bias": pc["b2"], "W": pc["Wv"], "avs": pc["asv"], "avd": pc["adv"]}
    return {"tfull": tfull, "adst": adst, "srcoff": pc["srcoff3"],
            "adstoff": pc["adstoff3"], "stidx": pc["stidx3"], "bias": pc["bv"]}


_CACHED = {}


def _get_programs(G12, G3, tmap12, tmap3):
    key = (G12, G3)
    if key not in _CACHED:
        progs = []
        for stage in range(4):
            G, tm = (G3, tmap3) if stage == 3 else (G12, tmap12)
            progs.append(_build_stage(stage, G, tm))
        _CACHED[key] = progs
    return _CACHED[key]


def run_pipeline(plan12, plan3, per_core, runner):
    """runner(nc, in_maps) -> (results list, exec_ns or None)"""
    progs = _get_programs(plan12["G"], plan3["G"],
                          plan12["tile_of_chunk"], plan3["tile_of_chunk"])
    tfull = None
    adst = [None] * NCORES
    outs = {}
    total_ns = 0
    have_ns = True
    for stage in range(4):
        in_maps = [_stage_inputs(stage, per_core[c], tfull, adst[c])
                   for c in range(NCORES)]
        results, ns = runner(progs[stage], in_maps)
        if ns is None:
            have_ns = False
        else:
            total_ns += ns
        if stage < 3:
            tfull = np.concatenate([results[c]["tloc_o"] for c in range(NCORES)],
                                   axis=0)
            adst = [results[c]["adst_o"] for c in range(NCORES)]
        if stage == 2:
            outs["mean"] = [results[c]["mean_o"] for c in range(NCORES)]
            outs["alpha"] = [results[c]["alpha_o"] for c in range(NCORES)]
        if stage == 3:
            outs["var"] = [results[c]["var_o"] for c in range(NCORES)]
    return outs, (total_ns if have_ns else None)


def _assemble(plan12, outs):
    NPC = _npc()
    mean = np.empty((N_NODES, CH), dtype=np.float32)
    var = np.empty((N_NODES, CH), dtype=np.float32)
    alpha = np.empty(N_EDGES, dtype=np.float32)
    for c in range(NCORES):
        mean[c * NPC:(c + 1) * NPC] = outs["mean"][c]
        var[c * NPC:(c + 1) * NPC] = outs["var"][c]
        pc = plan12["cores"][c]
        alpha[pc["orig"]] = outs["alpha"][c][pc["slot_p"], pc["slot_g"]]
    return mean, var, alpha


def _hw_runner(nc, in_maps, trace=False):
    from concourse.bass_utils import run_bass_kernel_spmd

    res = run_bass_kernel_spmd(nc, in_maps, list(range(NCORES)), trace=trace)
    return res.results, res.exec_time_ns


def kernel(**inputs):
    plan12, plan3, per_core = _prepare(inputs)
    outs, _ = run_pipeline(plan12, plan3, per_core, _hw_runner)
    return _assemble(plan12, outs)
